# revision 1
# baseline (speedup 1.0000x reference)
"""Causal single-head attention layer on 8 TRN2 NeuronCores.

Reference (per batch b):
  Q = x@Wq+bq; K = x@Wk+bk; V = x@Wv+bv        (S=4096, D=512, H=64)
  S = Q K^T / sqrt(S);  P = softmax(S + causal_mask);  out = (P V) @ Wo + bo

Sharding: 8 cores = 4 batches x 2 "halves". Each core owns 4 query-blocks
of 512 rows of its batch: even cores take blocks [7,4,3,0], odd take
[6,5,2,1] (causal work 72 k-tiles each). SPMD requires one program, so
both core types run the same *structural* schedule with per-slot k-tile
counts NKT=[32,24,16,8]; over-structural k-tiles are killed by per-core
mask data (an input tensor), so no collectives are needed.

On-chip algorithm per core (all matmuls bf16, fp32 PSUM accumulate):
  xT (D-on-partition x^T, host-pretransposed) -> K^T,V^T proj (stacked
  [Wk|Wv] stationary) and Q^T proj on a host-permuted xT_q.
  V^T -> V (natural) via PE transposes; V gets a ones column appended so
  the attention-weight row-sum (softmax denominator) falls out of the AV
  matmul for free.
  S^T tile [128k, 512q] = K^T_tile.T @ Q^T  -> exp (ACT, scale=1/64
  folded in) -> P^T bf16 -> mask-mul on last 4 groups -> AV accumulate
  out^T_aug [65, 512].  Final: y = (out^T_aug.T @ [Wo; bv@Wo+bo]) *
  (1/denom) per-partition; denom transposed to per-partition layout with
  a K=1 matmul. Softmax max-subtraction is skipped: |S/64| <~ 1 so exp
  is numerically safe.
"""

import os
import math

os.environ.setdefault("MYCRO_LOCAL_CACHE", "1")

import numpy as np
import ml_dtypes

import concourse.bass as bass
import concourse.mybir as mybir
import concourse.tile as tile
from concourse import bacc
from concourse.bass_utils import run_bass_kernel_spmd
from concourse.masks import make_identity

F32 = mybir.dt.float32
BF16 = mybir.dt.bfloat16

B, S, D, H = 4, 4096, 512, 64
QB = 512          # query block
NKT = [32, 24, 16, 8]   # structural k-tiles (of 128) per slot
BLOCKS_EVEN = [7, 4, 3, 0]
BLOCKS_ODD = [6, 5, 2, 1]

LAST_EXEC_TIME_NS = None
LAST_RESULTS = None


def _install_ntff_hook():
    """Register the axon NTFF profile hook if the image's antenv lacks it,
    so run_bass_kernel_spmd(trace=True) can report real exec_time_ns."""
    import sys
    import types
    try:
        from antenv.axon_hooks import get_axon_ntff_profile_hook  # noqa: F401
        return True  # already present
    except ImportError:
        pass
    try:
        import trn_agent_boot.trn_boot as _tb
        hook = _tb._ntff_profile_via_ctypes("/opt/axon/libaxon_pjrt.so")
        if hook is None:
            return False
        mod = types.ModuleType("antenv.axon_hooks")
        mod.get_axon_ntff_profile_hook = lambda: hook
        mod.set_axon_ntff_profile_hook = lambda h: None
        sys.modules["antenv.axon_hooks"] = mod
        return True
    except Exception:
        return False


def _build_nc():
    nc = bacc.Bacc(
        "TRN2",
        target_bir_lowering=False,
        debug=False,
        enable_asserts=False,
        num_devices=8,
    )

    xt_d = nc.dram_tensor("xt", [D, S], BF16, kind="ExternalInput")
    xtq_d = nc.dram_tensor("xtq", [D, 4 * QB], BF16, kind="ExternalInput")
    wkv_d = nc.dram_tensor("wkv", [D, 128], BF16, kind="ExternalInput")
    wq_d = nc.dram_tensor("wq", [D, H], BF16, kind="ExternalInput")
    wo_d = nc.dram_tensor("wo", [H + 1, D], BF16, kind="ExternalInput")
    bkv_d = nc.dram_tensor("bkv", [128, 1], F32, kind="ExternalInput")
    bq_d = nc.dram_tensor("bq", [H, 1], F32, kind="ExternalInput")
    mask_d = nc.dram_tensor("maskt", [4, 128, 4096], BF16, kind="ExternalInput")
    out_d = nc.dram_tensor("out", [4 * QB, D], F32, kind="ExternalOutput")

    krepeat = int(os.environ.get("KREPEAT", "1"))
    with tile.TileContext(nc) as tc:
      for _rep in range(krepeat):
        with (
            tc.tile_pool(name="big", bufs=1) as big,
            tc.tile_pool(name="small", bufs=1) as small,
        ):
            # ---- persistent SBUF tensors ----
            xt_sb = [big.tile([128, S], BF16, name=f"xt{j}", tag=f"xt{j}") for j in range(4)]
            xtq_sb = [big.tile([128, 4 * QB], BF16, name=f"xtq{j}", tag=f"xtq{j}") for j in range(4)]
            kvt_sb = big.tile([128, S], BF16, tag="kvt")      # rows 0:64 V^T, 64:128 K^T
            ktlo_sb = big.tile([64, S], BF16, tag="ktlo")     # K^T shifted to partitions 0:64
            qtp_sb = big.tile([64, 4 * QB], BF16, tag="qtp")  # Q^T permuted by slot
            vaug_sb = big.tile([128, 32 * 80], BF16, tag="vaug")
            mask_sb = big.tile([128, 4 * 4096], BF16, tag="mask")
            wkv_sb = small.tile([128, 4 * 128], BF16, tag="wkv")
            wq_sb = small.tile([128, 4 * H], BF16, tag="wq")
            wo_sb = small.tile([H + 1, D], BF16, tag="wo")
            bkv_sb = small.tile([128, 1], F32, tag="bkv")
            bq_sb = small.tile([H, 1], F32, tag="bq")
            ident_sb = small.tile([64, 64], BF16, tag="ident")
            ones_sb = small.tile([1, 1], BF16, tag="ones")

            # ---- input DMAs: weights on the fast HWDGE scalar ring first,
            # bulk xt on gpsimd SWDGE, masks last (needed latest) ----
            for j in range(4):
                nc.scalar.dma_start(
                    out=wkv_sb[:, j * 128:(j + 1) * 128],
                    in_=wkv_d[j * 128:(j + 1) * 128, :],
                )
            nc.scalar.dma_start(out=bkv_sb[:], in_=bkv_d[:, :])
            nc.scalar.dma_start(out=bq_sb[:], in_=bq_d[:, :])
            for j in range(4):
                nc.scalar.dma_start(
                    out=wq_sb[:, j * H:(j + 1) * H],
                    in_=wq_d[j * 128:(j + 1) * 128, :],
                )
            nc.scalar.dma_start(out=wo_sb[:], in_=wo_d[:, :])
            for j in range(4):
                for h in range(2):
                    nc.gpsimd.dma_start(
                        out=xt_sb[j][:, h * 2048:(h + 1) * 2048],
                        in_=xt_d[j * 128:(j + 1) * 128, h * 2048:(h + 1) * 2048],
                    )
                nc.scalar.dma_start(
                    out=xtq_sb[j][:],
                    in_=xtq_d[j * 128:(j + 1) * 128, :],
                )
            for s in range(4):
                nc.gpsimd.dma_start(
                    out=mask_sb[:, s * 4096:(s + 1) * 4096], in_=mask_d[s, :, :]
                )
            make_identity(nc, ident_sb[:])
            nc.vector.memset(ones_sb[:], 1.0)

            # ---- phase 1: projections ----
            with (
                tc.tile_pool(name="kvps", bufs=2, space="PSUM") as kvps,
                tc.tile_pool(name="qps", bufs=2, space="PSUM") as qps,
                tc.tile_pool(name="vtps", bufs=2, space="PSUM") as vtps,
            ):
                for sb in range(8):
                    kvp = kvps.tile([128, 512], F32, tag="kvp")
                    for j in range(4):
                        nc.tensor.matmul(
                            kvp[:],
                            lhsT=wkv_sb[:, j * 128:(j + 1) * 128],
                            rhs=xt_sb[j][:, sb * 512:(sb + 1) * 512],
                            start=(j == 0),
                            stop=(j == 3),
                        )
                    # V^T rows 0:64 get +0; K^T rows 64:128 get +bk
                    nc.vector.tensor_scalar_add(
                        kvt_sb[:, sb * 512:(sb + 1) * 512], kvp[:], bkv_sb[:]
                    )
                    nc.sync.dma_start(
                        out=ktlo_sb[:, sb * 512:(sb + 1) * 512],
                        in_=kvt_sb[64:128, sb * 512:(sb + 1) * 512],
                    )
                for qb in range(4):
                    qp = qps.tile([64, 512], F32, tag="qp")
                    for j in range(4):
                        nc.tensor.matmul(
                            qp[:],
                            lhsT=wq_sb[:, j * H:(j + 1) * H],
                            rhs=xtq_sb[j][:, qb * 512:(qb + 1) * 512],
                            start=(j == 0),
                            stop=(j == 3),
                        )
                    nc.vector.tensor_scalar_add(
                        qtp_sb[:, qb * 512:(qb + 1) * 512], qp[:], bq_sb[:]
                    )
                # V natural [128k, 64] tiles via PE transpose, plus ones col
                for kt in range(32):
                    vtp = vtps.tile([128, 64], BF16, tag="vtp")
                    nc.tensor.transpose(
                        vtp[:], kvt_sb[0:64, kt * 128:(kt + 1) * 128], ident_sb[:]
                    )
                    nc.vector.tensor_copy(
                        vaug_sb[:, kt * 80:kt * 80 + 64], vtp[:]
                    )
                vaug3 = vaug_sb[:].rearrange("p (k c) -> p k c", c=80)
                nc.vector.memset(vaug3[:, :, 64:65], 1.0)

            # ---- phase 2: attention ----
            with (
                tc.tile_pool(name="stps", bufs=2, space="PSUM") as stps,
                tc.tile_pool(name="otps", bufs=2, space="PSUM") as otps,
                tc.tile_pool(name="yps", bufs=1, space="PSUM") as yps,
                tc.tile_pool(name="dnps", bufs=1, space="PSUM") as dnps,
                tc.tile_pool(name="ptp", bufs=4) as ptp,
                tc.tile_pool(name="epi", bufs=2) as epi,
            ):
                for s in range(4):
                    nkt = NKT[s]
                    ngrp = nkt // 2
                    otp = otps.tile([65, 512], F32, tag="otp")
                    for g in range(ngrp):
                        stp = stps.tile([128, 1024], F32, tag="stp")
                        for u in range(2):
                            kt = 2 * g + u
                            nc.tensor.matmul(
                                stp[:, u * 512:(u + 1) * 512],
                                lhsT=ktlo_sb[:, kt * 128:(kt + 1) * 128],
                                rhs=qtp_sb[:, s * 512:(s + 1) * 512],
                                start=True,
                                stop=True,
                            )
                        pt = ptp.tile([128, 1024], BF16, tag="pt")
                        nc.scalar.activation(
                            pt[:], stp[:], mybir.ActivationFunctionType.Exp,
                            scale=1.0 / 64.0,
                        )
                        if g >= ngrp - 4:
                            gm = g - (ngrp - 4)
                            nc.gpsimd.tensor_mul(
                                pt[:], pt[:],
                                mask_sb[:, s * 4096 + gm * 1024: s * 4096 + (gm + 1) * 1024],
                            )
                        for u in range(2):
                            kt = 2 * g + u
                            nc.tensor.matmul(
                                otp[:],
                                lhsT=vaug_sb[:, kt * 80:kt * 80 + 65],
                                rhs=pt[:, u * 512:(u + 1) * 512],
                                start=(kt == 0),
                                stop=(kt == nkt - 1),
                            )
                    # epilogue for this slot
                    ot_sb = epi.tile([65, 512], BF16, tag="ot_sb")
                    dnrow = epi.tile([1, 512], BF16, tag="dnrow")
                    nc.vector.tensor_copy(ot_sb[:], otp[:])
                    nc.vector.tensor_copy(dnrow[:], otp[64:65, :])
                    for t in range(4):
                        dnp = dnps.tile([128, 1], F32, tag="dnp")
                        nc.tensor.matmul(
                            dnp[:],
                            lhsT=dnrow[:, t * 128:(t + 1) * 128],
                            rhs=ones_sb[:],
                            start=True,
                            stop=True,
                        )
                        recip = epi.tile([128, 1], F32, tag="recip")
                        nc.vector.reciprocal(recip[:], dnp[:])
                        yp = yps.tile([128, 512], F32, tag="yp")
                        nc.tensor.matmul(
                            yp[:],
                            lhsT=ot_sb[:, t * 128:(t + 1) * 128],
                            rhs=wo_sb[:],
                            start=True,
                            stop=True,
                        )
                        ysb = epi.tile([128, 512], F32, tag="ysb")
                        nc.vector.tensor_scalar_mul(ysb[:], yp[:], recip[:])
                        nc.sync.dma_start(
                            out=out_d[s * 512 + t * 128: s * 512 + (t + 1) * 128, :],
                            in_=ysb[:],
                        )

    nc.compile()
    return nc


_NC_CACHE = {}


def _tri_mask(r):
    # [128, 512] bf16: keep (1.0) where q_local >= 128*r + k_local
    i = np.arange(128)[:, None]
    j = np.arange(512)[None, :]
    return (j >= 128 * r + i).astype(ml_dtypes.bfloat16)


def _masks_for(blocks):
    m = np.zeros((4, 128, 4096), dtype=ml_dtypes.bfloat16)
    ones = np.ones((128, 512), dtype=ml_dtypes.bfloat16)
    zeros = np.zeros((128, 512), dtype=ml_dtypes.bfloat16)
    for s in range(4):
        nkt_a = 4 * (blocks[s] + 1)
        for g in range(4):
            for u in range(2):
                t = NKT[s] - 8 + 2 * g + u
                if t < nkt_a - 4:
                    tilem = ones
                elif t < nkt_a:
                    tilem = _tri_mask(t - (nkt_a - 4))
                else:
                    tilem = zeros
                c0 = g * 1024 + u * 512
                m[s, :, c0:c0 + 512] = tilem
    return m


def _make_in_maps(x, Wq, bq, Wk, bk, Wv, bv, Wo, bo):
    wkv = np.concatenate([Wv, Wk], axis=1).astype(ml_dtypes.bfloat16)  # (512, 128)
    bkv = np.concatenate([np.zeros(64, np.float32), bk])[:, None]
    wo_aug = np.concatenate([Wo, (bv @ Wo + bo)[None, :]], axis=0).astype(ml_dtypes.bfloat16)
    mask_even = _masks_for(BLOCKS_EVEN)
    mask_odd = _masks_for(BLOCKS_ODD)

    in_maps = []
    for c in range(8):
        b = c // 2
        blocks = BLOCKS_EVEN if c % 2 == 0 else BLOCKS_ODD
        xt = np.ascontiguousarray(x[b].T).astype(ml_dtypes.bfloat16)  # (512, 4096)
        qcols = np.concatenate(
            [np.arange(blk * QB, (blk + 1) * QB) for blk in blocks]
        )
        xtq = np.ascontiguousarray(xt[:, qcols])               # (512, 2048)
        in_maps.append({
            "xt": xt,
            "xtq": xtq,
            "wkv": wkv,
            "wq": Wq.astype(ml_dtypes.bfloat16),
            "wo": wo_aug,
            "bkv": bkv,
            "bq": bq[:, None],
            "maskt": mask_even if c % 2 == 0 else mask_odd,
        })
    return in_maps


def kernel(x, Wq, bq, Wk, bk, Wv, bv, Wo, bo):
    global LAST_EXEC_TIME_NS, LAST_RESULTS
    x = np.asarray(x, dtype=np.float32)
    Wq, bq = np.asarray(Wq, np.float32), np.asarray(bq, np.float32)
    Wk, bk = np.asarray(Wk, np.float32), np.asarray(bk, np.float32)
    Wv, bv = np.asarray(Wv, np.float32), np.asarray(bv, np.float32)
    Wo, bo = np.asarray(Wo, np.float32), np.asarray(bo, np.float32)

    if "nc" not in _NC_CACHE:
        _NC_CACHE["nc"] = _build_nc()
    nc = _NC_CACHE["nc"]

    in_maps = _make_in_maps(x, Wq, bq, Wk, bk, Wv, bv, Wo, bo)

    trace = os.environ.get("KERNEL_TRACE", "1") == "1"
    if trace:
        trace = _install_ntff_hook()
    tmpdir = os.environ.get("KERNEL_TRACE_DIR") or None
    try:
        res = run_bass_kernel_spmd(
            nc, in_maps, core_ids=list(range(8)), trace=trace, tmpdir=tmpdir
        )
    except Exception:
        if not trace:
            raise
        res = run_bass_kernel_spmd(nc, in_maps, core_ids=list(range(8)), trace=False)
    LAST_EXEC_TIME_NS = res.exec_time_ns
    LAST_RESULTS = res

    out = np.empty((B, S, D), np.float32)
    for c in range(8):
        b = c // 2
        blocks = BLOCKS_EVEN if c % 2 == 0 else BLOCKS_ODD
        shard = res.results[c]["out"]
        for sidx, blk in enumerate(blocks):
            out[b, blk * QB:(blk + 1) * QB, :] = shard[sidx * QB:(sidx + 1) * QB, :]
    return out



# revision 9
# speedup vs baseline: 1.0495x; 1.0495x over previous
"""Causal single-head attention layer on 8 TRN2 NeuronCores.

Reference (per batch b):
  Q = x@Wq+bq; K = x@Wk+bk; V = x@Wv+bv        (S=4096, D=512, H=64)
  S = Q K^T / sqrt(S);  P = softmax(S + causal_mask);  out = (P V) @ Wo + bo

Sharding: 8 cores = 4 batches x 2 halves. Each core owns 4 query-blocks
of 512 rows of its batch in ASCENDING causal order: even cores take
blocks [0,3,4,7], odd take [1,2,5,6]. SPMD structural k-tile counts per
slot NKT=[8,16,24,32] cover both parities; over-structural k-tiles and
the causal boundary are killed by an additive -1e5 mask generated
ON-CHIP from an iota ramp compared against a per-core threshold input
(thr[s] = (NKT[s]-8)*128 - 512*blk_s) -- no mask DMA.

Schedule (single tensor-engine stream, emission order = execution
order): KV-projection chunks, Q-projection, attention slots and
epilogues are interleaved so slot s runs as soon as k-tiles 0..NKT[s]-1
exist. Ascending slot order makes slot 0 ready after only 1/4 of the
K/V projection. Inputs stream over 3 DMA queues (scalar/sync/gpsimd).

Per group of 2 k-tiles: S^T [128k,1024] = K^T.T @ Q^T (bf16 PE) ->
(+mask, DVE, tail groups only) -> exp via ACT (scale=1/64 folded) ->
P^T bf16 -> AV accumulate otp[65,512] (V_aug carries a ones column so
the softmax denominator falls out of row 64). QK/AV are software-
pipelined (QK(g+1) emitted before AV(g)) so the PE never waits on the
exp. Epilogue: recip of the denominator ROW (DVE) -> scale ot by
partition-broadcast -> append ones row -> y = ot_n^T @ [Wo; bv@Wo+bo]
-> bf16 out DMA (host casts to f32). Softmax max-subtraction skipped:
|S/64| <~ 1 so exp is numerically safe.
"""

import os

os.environ.setdefault("MYCRO_LOCAL_CACHE", "1")

import numpy as np
import ml_dtypes

import concourse.bass as bass
import concourse.mybir as mybir
import concourse.tile as tile
from concourse import bacc
from concourse.bass_utils import run_bass_kernel_spmd
from concourse.masks import make_identity

F32 = mybir.dt.float32
BF16 = mybir.dt.bfloat16

B, S, D, H = 4, 4096, 512, 64
QB = 512
NKT = [8, 16, 24, 32]          # structural k-tiles per slot (ascending)
BLOCKS_EVEN = [0, 3, 4, 7]
BLOCKS_ODD = [1, 2, 5, 6]

LAST_EXEC_TIME_NS = None
LAST_RESULTS = None


def _install_ntff_hook():
    import sys
    import types
    try:
        from antenv.axon_hooks import get_axon_ntff_profile_hook  # noqa: F401
        return True
    except ImportError:
        pass
    try:
        import trn_agent_boot.trn_boot as _tb
        hook = _tb._ntff_profile_via_ctypes("/opt/axon/libaxon_pjrt.so")
        if hook is None:
            return False
        mod = types.ModuleType("antenv.axon_hooks")
        mod.get_axon_ntff_profile_hook = lambda: hook
        mod.set_axon_ntff_profile_hook = lambda h: None
        sys.modules["antenv.axon_hooks"] = mod
        return True
    except Exception:
        return False


def _build_nc():
    nc = bacc.Bacc(
        "TRN2",
        target_bir_lowering=False,
        debug=False,
        enable_asserts=False,
        num_devices=8,
    )

    xt_d = nc.dram_tensor("xt", [D, S], BF16, kind="ExternalInput")
    xtq_d = nc.dram_tensor("xtq", [D, 4 * QB], BF16, kind="ExternalInput")
    wkv_d = nc.dram_tensor("wkv", [D, 128], BF16, kind="ExternalInput")
    wq_d = nc.dram_tensor("wq", [D, H], BF16, kind="ExternalInput")
    wo_d = nc.dram_tensor("wo", [H + 1, D], BF16, kind="ExternalInput")
    bkv_d = nc.dram_tensor("bkv", [128, 1], F32, kind="ExternalInput")
    bq_d = nc.dram_tensor("bq", [H, 1], F32, kind="ExternalInput")
    thr_d = nc.dram_tensor("thr", [128, 4], F32, kind="ExternalInput")
    out_d = nc.dram_tensor("out", [4 * QB, D], BF16, kind="ExternalOutput")

    with tile.TileContext(nc) as tc:
        with (
            tc.tile_pool(name="big", bufs=1) as big,
            tc.tile_pool(name="small", bufs=1) as small,
            tc.tile_pool(name="projps", bufs=2, space="PSUM") as projps,
            tc.tile_pool(name="stps", bufs=2, space="PSUM") as stps,
            tc.tile_pool(name="otps", bufs=2, space="PSUM") as otps,
            tc.tile_pool(name="ptp", bufs=3) as ptp,
            tc.tile_pool(name="epi", bufs=2) as epi,
        ):
            # ---- persistent SBUF ----
            xt_sb = [big.tile([128, S], BF16, name=f"xt{j}", tag=f"xt{j}") for j in range(4)]
            xtq_sb = [big.tile([128, 4 * QB], BF16, name=f"xtq{j}", tag=f"xtq{j}") for j in range(4)]
            kvt_sb = big.tile([128, S], BF16, tag="kvt")      # rows 0:64 V^T, 64:128 K^T
            ktlo_sb = big.tile([64, S], BF16, tag="ktlo")     # K^T at partitions 0:64
            qtp_sb = big.tile([64, 4 * QB], BF16, tag="qtp")  # Q^T slot-ordered
            vaug_sb = big.tile([128, 32 * 80], BF16, tag="vaug")
            iota_sb = big.tile([128, 8, 512], F32, tag="iota")
            mask_sb = big.tile([128, 4, 4096], BF16, tag="mask")
            wkv_sb = small.tile([128, 4 * 128], BF16, tag="wkv")
            wq_sb = small.tile([128, 4 * H], BF16, tag="wq")
            wo_sb = small.tile([H + 1, D], BF16, tag="wo")
            bkv_sb = small.tile([128, 1], F32, tag="bkv")
            bq_sb = small.tile([H, 1], F32, tag="bq")
            thr_sb = small.tile([128, 4], F32, tag="thr")
            ident_sb = small.tile([64, 64], BF16, tag="ident")
            ones_sb = small.tile([1, 1], BF16, tag="ones")

            vaug3 = vaug_sb[:].rearrange("p (k c) -> p k c", c=80)

            # ---- input DMAs ----
            # weights/biases/thresholds first on the scalar ring
            for j in range(4):
                nc.scalar.dma_start(
                    out=wkv_sb[:, j * 128:(j + 1) * 128],
                    in_=wkv_d[j * 128:(j + 1) * 128, :],
                )
            for j in range(4):
                nc.scalar.dma_start(
                    out=wq_sb[:, j * H:(j + 1) * H],
                    in_=wq_d[j * 128:(j + 1) * 128, :],
                )
            nc.scalar.dma_start(out=wo_sb[:], in_=wo_d[:, :])
            nc.scalar.dma_start(out=bkv_sb[:], in_=bkv_d[:, :])
            nc.scalar.dma_start(out=bq_sb[:], in_=bq_d[:, :])
            nc.scalar.dma_start(out=thr_sb[:], in_=thr_d[:, :])
            # xt: 16 column-chunks [128,1024]; column-major order so the
            # first k-tiles complete first; spread j over 3 queues.
            xt_q = [nc.scalar, nc.sync, nc.gpsimd, nc.sync]
            for c in range(4):
                for j in range(4):
                    xt_q[j].dma_start(
                        out=xt_sb[j][:, c * 1024:(c + 1) * 1024],
                        in_=xt_d[j * 128:(j + 1) * 128, c * 1024:(c + 1) * 1024],
                    )
            # xtq: blocks 0+1 first (needed by slots 0/1), then 2+3
            for c in range(2):
                for j in range(4):
                    xt_q[(j + 1) % 4].dma_start(
                        out=xtq_sb[j][:, c * 1024:(c + 1) * 1024],
                        in_=xtq_d[j * 128:(j + 1) * 128, c * 1024:(c + 1) * 1024],
                    )

            make_identity(nc, ident_sb[:])
            nc.vector.memset(vaug3[:, :, 64:65], 1.0)
            nc.vector.memset(ones_sb[:], 1.0)
            # iota ramp v[p,t,j] = -128t + j - p (f32 exact for small ints)
            nc.gpsimd.iota(
                iota_sb[:], pattern=[[-128, 8], [1, 512]], base=0,
                channel_multiplier=-1, allow_small_or_imprecise_dtypes=True,
            )

            def emit_mask(s):
                # mask[p,t,j] = -1e5 where (-128t + j - p) < thr[s] else 0
                nc.vector.tensor_scalar(
                    out=mask_sb[:, s, :],
                    in0=iota_sb[:].rearrange("p a b -> p (a b)"),
                    scalar1=thr_sb[:, s:s + 1],
                    scalar2=-100000.0,
                    op0=mybir.AluOpType.is_lt,
                    op1=mybir.AluOpType.mult,
                )

            def emit_kv_chunk(c):
                # seq cols [c*1024,(c+1)*1024) = k-tiles 8c..8c+7
                for half in range(2):
                    col = c * 1024 + half * 512
                    kvp = projps.tile([128, 512], F32, tag="proj")
                    for j in range(4):
                        nc.tensor.matmul(
                            kvp[:],
                            lhsT=wkv_sb[:, j * 128:(j + 1) * 128],
                            rhs=xt_sb[j][:, col:col + 512],
                            start=(j == 0),
                            stop=(j == 3),
                        )
                    nc.vector.tensor_scalar_add(
                        kvt_sb[:, col:col + 512], kvp[:], bkv_sb[:]
                    )
                nc.sync.dma_start(
                    out=ktlo_sb[:, c * 1024:(c + 1) * 1024],
                    in_=kvt_sb[64:128, c * 1024:(c + 1) * 1024],
                )
                for kt in range(8 * c, 8 * c + 8):
                    vtp = projps.tile([128, 64], BF16, tag="proj")
                    nc.tensor.transpose(
                        vtp[:], kvt_sb[0:64, kt * 128:(kt + 1) * 128], ident_sb[:]
                    )
                    nc.vector.tensor_copy(vaug3[:, kt, 0:64], vtp[:])

            def emit_q_chunk(c):
                for blk in (2 * c, 2 * c + 1):
                    qp = projps.tile([128, 512], F32, tag="proj")
                    for j in range(4):
                        nc.tensor.matmul(
                            qp[0:64, :],
                            lhsT=wq_sb[:, j * H:(j + 1) * H],
                            rhs=xtq_sb[j][:, blk * 512:(blk + 1) * 512],
                            start=(j == 0),
                            stop=(j == 3),
                        )
                    nc.vector.tensor_scalar_add(
                        qtp_sb[:, blk * 512:(blk + 1) * 512], qp[0:64, :], bq_sb[:]
                    )

            def emit_slot(s):
                nkt = NKT[s]
                ngrp = nkt // 2
                otp = otps.tile([H + 1, 512], F32, tag="otp")
                pts = {}
                for g in range(ngrp + 1):
                    if g < ngrp:
                        stp = stps.tile([128, 1024], F32, tag="stp")
                        for u in range(2):
                            kt = 2 * g + u
                            nc.tensor.matmul(
                                stp[:, u * 512:(u + 1) * 512],
                                lhsT=ktlo_sb[:, kt * 128:(kt + 1) * 128],
                                rhs=qtp_sb[:, s * 512:(s + 1) * 512],
                                start=True,
                                stop=True,
                            )
                        if g >= ngrp - 4:
                            gm = g - (ngrp - 4)
                            nc.vector.tensor_add(
                                stp[:],
                                stp[:],
                                mask_sb[:, s, gm * 1024:(gm + 1) * 1024],
                            )
                        pt = ptp.tile([128, 1024], BF16, tag="pt")
                        nc.scalar.activation(
                            pt[:], stp[:], mybir.ActivationFunctionType.Exp,
                            scale=1.0 / 64.0,
                        )
                        pts[g] = pt
                    if g >= 1:
                        ptm = pts.pop(g - 1)
                        for u in range(2):
                            kt = 2 * (g - 1) + u
                            nc.tensor.matmul(
                                otp[:],
                                lhsT=vaug3[:, kt, 0:65],
                                rhs=ptm[:, u * 512:(u + 1) * 512],
                                start=(kt == 0),
                                stop=(kt == nkt - 1),
                            )
                # epilogue A: stash ot (incl. denominator row 64) in bf16;
                # dnrow is a partition-0 copy of the denominator row for the
                # K=1 transpose matmuls
                ot_sb = epi.tile([H + 1, 512], BF16, tag="ot_sb")
                dnrow = epi.tile([1, 512], BF16, tag="dnrow")
                nc.vector.tensor_copy(ot_sb[:], otp[:])
                nc.vector.tensor_copy(dnrow[:], otp[H:H + 1, :])
                return ot_sb, dnrow

            def emit_epi_b(s, ot_sb, dnrow):
                for t in range(4):
                    dnp = projps.tile([128, 1], F32, tag="proj")
                    nc.tensor.matmul(
                        dnp[:],
                        lhsT=dnrow[:, t * 128:(t + 1) * 128],
                        rhs=ones_sb[:],
                        start=True,
                        stop=True,
                    )
                    recip = epi.tile([128, 1], F32, tag="recip")
                    nc.vector.reciprocal(recip[:], dnp[:])
                    yp = projps.tile([128, 512], F32, tag="proj")
                    nc.tensor.matmul(
                        yp[:],
                        lhsT=ot_sb[:, t * 128:(t + 1) * 128],
                        rhs=wo_sb[:],
                        start=True,
                        stop=True,
                    )
                    ysb = epi.tile([128, 512], BF16, tag="ysb")
                    nc.vector.tensor_scalar_mul(ysb[:], yp[:], recip[:])
                    nc.sync.dma_start(
                        out=out_d[s * 512 + t * 128:s * 512 + (t + 1) * 128, :],
                        in_=ysb[:],
                    )

            emit_kv_chunk(0)
            emit_q_chunk(0)
            emit_mask(0)
            ot0 = emit_slot(0)
            emit_kv_chunk(1)
            emit_mask(1)
            emit_epi_b(0, *ot0)
            ot1 = emit_slot(1)
            emit_kv_chunk(2)
            emit_q_chunk(1)
            emit_mask(2)
            emit_epi_b(1, *ot1)
            ot2 = emit_slot(2)
            emit_kv_chunk(3)
            emit_mask(3)
            emit_epi_b(2, *ot2)
            ot3 = emit_slot(3)
            emit_epi_b(3, *ot3)

    nc.compile()
    return nc


_NC_CACHE = {}


def _make_in_maps(x, Wq, bq, Wk, bk, Wv, bv, Wo, bo):
    wkv = np.concatenate([Wv, Wk], axis=1).astype(ml_dtypes.bfloat16)  # (512, 128)
    bkv = np.concatenate([np.zeros(64, np.float32), bk])[:, None]
    wo_aug = np.concatenate([Wo, (bv @ Wo + bo)[None, :]], axis=0).astype(ml_dtypes.bfloat16)

    in_maps = []
    for c in range(8):
        b = c // 2
        blocks = BLOCKS_EVEN if c % 2 == 0 else BLOCKS_ODD
        xt = np.ascontiguousarray(x[b].T).astype(ml_dtypes.bfloat16)  # (512, 4096)
        qcols = np.concatenate(
            [np.arange(blk * QB, (blk + 1) * QB) for blk in blocks]
        )
        xtq = np.ascontiguousarray(xt[:, qcols])               # (512, 2048)
        thr = np.zeros((128, 4), np.float32)
        for s in range(4):
            thr[:, s] = (NKT[s] - 8) * 128 - 512 * blocks[s]
        in_maps.append({
            "xt": xt,
            "xtq": xtq,
            "wkv": wkv,
            "wq": Wq.astype(ml_dtypes.bfloat16),
            "wo": wo_aug,
            "bkv": bkv,
            "bq": bq[:, None],
            "thr": thr,
        })
    return in_maps


def kernel(x, Wq, bq, Wk, bk, Wv, bv, Wo, bo):
    global LAST_EXEC_TIME_NS, LAST_RESULTS
    x = np.asarray(x, dtype=np.float32)
    Wq, bq = np.asarray(Wq, np.float32), np.asarray(bq, np.float32)
    Wk, bk = np.asarray(Wk, np.float32), np.asarray(bk, np.float32)
    Wv, bv = np.asarray(Wv, np.float32), np.asarray(bv, np.float32)
    Wo, bo = np.asarray(Wo, np.float32), np.asarray(bo, np.float32)

    if "nc" not in _NC_CACHE:
        _NC_CACHE["nc"] = _build_nc()
    nc = _NC_CACHE["nc"]

    in_maps = _make_in_maps(x, Wq, bq, Wk, bk, Wv, bv, Wo, bo)

    trace = os.environ.get("KERNEL_TRACE", "1") == "1"
    if trace:
        trace = _install_ntff_hook()
    tmpdir = os.environ.get("KERNEL_TRACE_DIR") or None
    try:
        res = run_bass_kernel_spmd(
            nc, in_maps, core_ids=list(range(8)), trace=trace, tmpdir=tmpdir
        )
    except Exception:
        if not trace:
            raise
        res = run_bass_kernel_spmd(nc, in_maps, core_ids=list(range(8)), trace=False)
    LAST_EXEC_TIME_NS = res.exec_time_ns
    LAST_RESULTS = res

    out = np.empty((B, S, D), np.float32)
    for c in range(8):
        b = c // 2
        blocks = BLOCKS_EVEN if c % 2 == 0 else BLOCKS_ODD
        shard = res.results[c]["out"].astype(np.float32)
        for sidx, blk in enumerate(blocks):
            out[b, blk * QB:(blk + 1) * QB, :] = shard[sidx * QB:(sidx + 1) * QB, :]
    return out
